# revision 58
# baseline (speedup 1.0000x reference)
"""Single-head causal attention on 8 TRN2 NeuronCores.

Problem: x[8, 2048, 1024] f32; Wq/Wk/Wv[1024, 128]; bq/bk/bv[128].
  q = x@Wq+bq; k = x@Wk+bk; v = x@Wv+bv
  scores[b,t,s] = k[b,t,:].q[b,s,:] / sqrt(128), causal (s<=t), softmax over s
  out = weights @ v   -> [8, 2048, 128] f32

Sharding: data-parallel over batch, one batch element per core. No collectives.

Per-core algorithm (T=2048, D=1024, H=128), matmuls in bf16. Design notes:
  - scores computed TRANSPOSED: S_T[s, t] = qT.T @ kT, so P_T = exp(S_T) is
    directly the stationary operand of out[t, 129] = P_T.T @ v_aug; the ones
    column of v_aug yields the softmax denominator for free.
  - bv is folded into the v rows (v' = v + bv): softmax weights sum to 1, so
    out/denom + bv == (P@(v+bv))/denom.  Kills the epilogue bias add.
  - x is loaded as dc-pair chunks [128, 2048] (4KB rows: descriptors below
    4KB run at reduced HBM efficiency), B column-half first; DMA triggers
    split between the SP and ACT hardware DGE queues (each trigger costs
    ~600-700ns of queue time) plus gpsimd SWDGE for half the output tiles.
  - ONE set of PSUM pools lives for the whole kernel (no pool-handoff
    barriers): proj ring (2 banks) + S ring (3) + v/O accumulator ring (3);
    O tiles also recycle the proj ring once projections finish.
  - schedule: G1 k3+v15/v14 track the x pair DMAs, q3 chain after (wq lands
    late); G2 S rows 15..8 at their earliest deps (k3/q3, then k2/q2) keep
    the ACT exp stream fed from ~22us; G3 runs all four A-half proj chains
    the moment the A half lands, with rows 0..3's high chunks dropping in
    as q0/k1 resolve; G4 finishes rows 0..3 (diag) then rows 4..7 -- the
    last gates -- with v-row chains as PE filler between exp-paced S
    matmuls, then O chains t15..t12 pre-run all but their si 4..7 tails.
  - O accumulation chains order si so chains end on rows 7..4 (exp'd last);
    epilogue = DVE reciprocal + DVE/ACT multiply, out-DMAs alternate
    SP/gpsimd queues; tiny chains t3..t0 run early in the finale and the
    final tile takes the fastest epilogue path (DVE + SP).
"""

import math

import ml_dtypes
import numpy as np

import concourse.bass as bass
import concourse.mybir as mybir
import concourse.tile as tile
from concourse import bacc
from concourse.bass_utils import run_bass_kernel_spmd

B, T, D, H = 8, 2048, 1024, 128
NT = T // 128          # 16 t/s tiles
ND = D // 128          # 8 contraction chunks
SCALE = 1.0 / math.sqrt(H)

F32 = mybir.dt.float32
BF16 = mybir.dt.bfloat16
AF = mybir.ActivationFunctionType


def build_nc():
    nc = bacc.Bacc(
        "TRN2",
        target_bir_lowering=False,
        debug=False,
        num_devices=8,
    )

    # x[b].T split into 4 dc-pair chunks x 2 column halves: B half = cols
    # 1024:2048 (consumed first), A half = cols 0:1024. Pair chunk j holds
    # d-chunks 2j and 2j+1 side by side: [128, 2048] with 4KB contiguous
    # rows (DMA descriptors below 4KB run at reduced HBM efficiency).
    xb_d = nc.dram_tensor("xb", [4, 128, 2048], BF16, kind="ExternalInput")
    xa_d = nc.dram_tensor("xa", [4, 128, 2048], BF16, kind="ExternalInput")
    w_d = {
        p: nc.dram_tensor(f"w{p}", [128, ND, H], BF16, kind="ExternalInput")
        for p in ("q", "k", "v")
    }
    bias_d = nc.dram_tensor("bias", [H, 2], F32, kind="ExternalInput")
    mask_d = nc.dram_tensor("mask", [128, 128], BF16, kind="ExternalInput")
    bvb_d = nc.dram_tensor("bvb", [128, 128], F32, kind="ExternalInput")
    out_d = nc.dram_tensor("out", [T, H], F32, kind="ExternalOutput")

    with tile.TileContext(nc) as tc:
        with (
            tc.tile_pool(name="const", bufs=1) as const_pool,
            tc.tile_pool(name="x", bufs=1) as x_pool,
            tc.tile_pool(name="qk", bufs=1) as qk_pool,
            tc.tile_pool(name="vrows", bufs=1) as v_pool,
            tc.tile_pool(name="prows", bufs=1) as p_pool,
            tc.tile_pool(name="eps", bufs=3) as ep_pool,
            tc.tile_pool(name="projps", bufs=2, space="PSUM") as proj_ps,
            tc.tile_pool(name="sps", bufs=3, space="PSUM") as s_ps_pool,
            tc.tile_pool(name="accps", bufs=3, space="PSUM") as acc_ps,
        ):
            w_sb = {}
            for p in ("q", "k", "v"):
                w_sb[p] = const_pool.tile(
                    [128, ND, H], BF16, tag=f"w{p}", name=f"w{p}_sb"
                )
            bias_sb = const_pool.tile([128, 2], F32, tag="bias")
            mask_sb = const_pool.tile([128, 128], BF16, tag="mask")
            bvb_sb = const_pool.tile([128, 128], F32, tag="bvb")
            xh = {
                "b": [x_pool.tile([128, 2048], BF16, tag=f"xb{j}", name=f"xb{j}_sb")
                      for j in range(4)],
                "a": [x_pool.tile([128, 2048], BF16, tag=f"xa{j}", name=f"xa{j}_sb")
                      for j in range(4)],
            }

            def x_ap(half, dc, off, w):
                # column slice [off, off+w) of d-chunk dc within the half
                return xh[half][dc // 2][:, (dc % 2) * 1024 + off : (dc % 2) * 1024 + off + w]

            # ---- input DMAs ----
            # Transfers drain roughly in program order: x pair 0 first so the
            # first projection matmuls start ASAP; weights interleaved right
            # behind; A half after B half (not needed until ~mid-kernel).
            nc.scalar.dma_start(w_sb["k"][:], w_d["k"][:])
            nc.scalar.dma_start(w_sb["v"][:], w_d["v"][:])
            nc.sync.dma_start(xh["b"][0][:], xb_d[0, :, :])
            nc.sync.dma_start(xh["b"][1][:], xb_d[1, :, :])
            nc.scalar.dma_start(w_sb["q"][:], w_d["q"][:])
            nc.scalar.dma_start(bias_sb[:], bias_d[:])
            nc.sync.dma_start(xh["b"][2][:], xb_d[2, :, :])
            nc.scalar.dma_start(bvb_sb[:], bvb_d[:])
            nc.scalar.dma_start(mask_sb[:], mask_d[:])
            nc.sync.dma_start(xh["b"][3][:], xb_d[3, :, :])
            for j in range(4):
                nc.sync.dma_start(xh["a"][j][:], xa_d[j, :, :])
            # pre-warm the ACT exp table while DMAs land
            warm = const_pool.tile([128, 1], F32, tag="warm")
            nc.scalar.activation(warm[:], bias_sb[:, 0:1], AF.Exp, scale=0.0)
            # pre-warm the PE p-state during the DMA wait: the memset is
            # dependency-free (hoisted into the preamble), so this dummy
            # matmul chain runs from ~7.5us until the first x chunk lands,
            # ramping the PE clock before the real projections start.
            wmm_in = x_pool.tile([128, 512], BF16, tag="wmm", name="warm_mm_in")
            nc.vector.memset(wmm_in[:], 0.0)
            wmm_ps = s_ps_pool.tile([128, 512], F32, tag="sps", name="warm_mm_ps")
            def warm_mms(n):
                for r in range(n):
                    nc.tensor.matmul(
                        wmm_ps[:, 0:128],
                        wmm_in[:, 0:128],
                        wmm_in[:, 0:128],
                        start=(r == 0),
                        stop=(r == n - 1),
                    )

            warm_mms(44)

            qk_sb = {"q": [None] * 4, "k": [None] * 4}
            v_rows = [None] * NT
            p_rows = [None] * NT

            def proj_chunk(p, ncol):
                """8-dc accumulation chain for projection p, global cols
                [512*ncol, 512*ncol+512); returns the psum tile."""
                half = "b" if ncol >= 2 else "a"
                off = (ncol % 2) * 512
                ps = proj_ps.tile([128, 512], F32, tag="proj", name=f"ps_{p}{ncol}")
                for dc in range(ND):
                    nc.tensor.matmul(
                        ps[:],
                        w_sb[p][:, dc, :],
                        x_ap(half, dc, off, 512),
                        start=(dc == 0),
                        stop=(dc == ND - 1),
                    )
                return ps

            def proj_copy(p, ncol, ps):
                sb_t = qk_pool.tile(
                    [128, 512], BF16, tag=f"{p}{ncol}", name=f"{p}T{ncol}_sb"
                )
                bi = 0 if p == "q" else 1
                nc.vector.tensor_scalar_add(sb_t[:], ps[:], bias_sb[:, bi : bi + 1])
                qk_sb[p][ncol] = sb_t

            def v_finish(si, vp):
                vr = v_pool.tile([128, 129], BF16, tag=f"v{si}", name=f"v{si}_sb")
                nc.vector.tensor_add(vr[:, 0:128], vp[:, 0:128], bvb_sb[:])
                nc.vector.memset(vr[:, 128:129], 1.0)
                v_rows[si] = vr

            def v_row(si):
                vp = acc_ps.tile([128, 129], F32, tag="acc", name=f"v_ps{si}")
                half = "b" if si >= 8 else "a"
                for dc in range(ND):
                    nc.tensor.matmul(
                        vp[:, 0:128],
                        x_ap(half, dc, (si % 8) * 128, 128),
                        w_sb["v"][:, dc, :],
                        start=(dc == 0),
                        stop=(dc == ND - 1),
                    )
                v_finish(si, vp)

            def s_chunks(si, c_lo, c_hi, alloc_pr=False):
                """S row si, global cols [c_lo, c_hi): matmul + exp per
                512-aligned chunk; mask if the diagonal chunk is included."""
                gc0 = si * 128
                if alloc_pr:
                    p_rows[si] = p_pool.tile(
                        [128, T - gc0], BF16, tag=f"p{si}", name=f"p{si}_sb"
                    )
                pr = p_rows[si]
                c = c_lo
                while c < c_hi:
                    ce = min(c_hi, (c // 512 + 1) * 512)
                    s_ps = s_ps_pool.tile(
                        [128, 512], F32, tag="sps", name=f"s_ps_{si}_{c}"
                    )
                    nc.tensor.matmul(
                        s_ps[:, 0 : ce - c],
                        qk_sb["q"][si // 4][:, (si % 4) * 128 : (si % 4 + 1) * 128],
                        qk_sb["k"][c // 512][:, c % 512 : c % 512 + (ce - c)],
                        start=True,
                        stop=True,
                    )
                    nc.scalar.activation(
                        pr[:, c - gc0 : ce - gc0],
                        s_ps[:, 0 : ce - c],
                        AF.Exp,
                        scale=SCALE,
                    )
                    c = ce
                if c_lo == gc0:  # diagonal chunk: causal mask (keep s <= t)
                    nc.vector.tensor_mul(pr[:, 0:128], pr[:, 0:128], mask_sb[:])

            def s_row(si):
                s_chunks(si, si * 128, T, alloc_pr=True)

            # ---- G1: {k3,k2} proj + v15/v14 track the xb pair arrivals
            # (~1.3us of PE work per ~1.4us pair: dense enough to hold the
            # PE p-state at full clock). wk/wv land first; wq lands 5th, so
            # the q chains run contiguously after, on the freed k slots. ----
            ps_k3 = proj_ps.tile([128, 512], F32, tag="proj", name="ps_k3")
            ps_k2 = proj_ps.tile([128, 512], F32, tag="proj", name="ps_k2")
            vp1 = {
                si: acc_ps.tile([128, 129], F32, tag="acc", name=f"v_ps{si}")
                for si in (15, 14)
            }
            for j in range(4):
                pair = (2 * j, 2 * j + 1)
                for dc in pair:
                    nc.tensor.matmul(
                        ps_k3[:], w_sb["k"][:, dc, :], x_ap("b", dc, 512, 512),
                        start=(dc == 0), stop=(dc == ND - 1),
                    )
                for dc in pair:
                    nc.tensor.matmul(
                        ps_k2[:], w_sb["k"][:, dc, :], x_ap("b", dc, 0, 512),
                        start=(dc == 0), stop=(dc == ND - 1),
                    )
                for si in (15, 14):
                    for dc in pair:
                        nc.tensor.matmul(
                            vp1[si][:, 0:128],
                            x_ap("b", dc, (si % 8) * 128, 128),
                            w_sb["v"][:, dc, :],
                            start=(dc == 0),
                            stop=(dc == ND - 1),
                        )
                # dependency-free filler holds the PE clock through the
                # wait for the next x pair DMA
                warm_mms(8)
            proj_copy("k", 3, ps_k3)
            proj_copy("k", 2, ps_k2)
            for si in (15, 14):
                v_finish(si, vp1[si])

            # ---- G2: q3/q2 chains on the freed slots; S rows 15..8 at
            # their earliest deps feed ACT's exp stream from ~19us ----
            ps_q3 = proj_chunk("q", 3)
            proj_copy("q", 3, ps_q3)
            s_row(15)
            s_row(14)
            ps_q2 = proj_chunk("q", 2)
            s_row(13)
            proj_copy("q", 2, ps_q2)
            s_row(12)
            v_row(13)
            s_row(11)
            v_row(12)
            s_row(10)
            v_row(11)
            s_row(9)
            v_row(10)
            s_row(8)

            # ---- G3: A-half proj chains the moment the A half lands; S
            # chunks of rows 0..3 drop in as soon as their q0/k1/k0 deps
            # resolve, keeping ACT's exp stream continuous; v rows fill ----
            ps_q0 = proj_chunk("q", 0)
            proj_copy("q", 0, ps_q0)
            ps_k1 = proj_chunk("k", 1)
            s_chunks(0, 1024, T, alloc_pr=True)
            s_chunks(1, 1024, T, alloc_pr=True)
            proj_copy("k", 1, ps_k1)
            ps_q1 = proj_chunk("q", 1)
            s_chunks(2, 1024, T, alloc_pr=True)
            s_chunks(3, 1024, T, alloc_pr=True)
            proj_copy("q", 1, ps_q1)
            ps_k0 = proj_chunk("k", 0)
            s_chunks(0, 512, 1024)
            v_row(9)
            s_chunks(1, 512, 1024)
            s_chunks(2, 512, 1024)
            proj_copy("k", 0, ps_k0)
            v_row(8)
            s_chunks(3, 512, 1024)

            # ---- O phase setup: per t-tile accumulation chains. Chain
            # order ends on rows 7..4 (exp'd last); O tiles alternate
            # between the acc ring and the now-free proj ring. Chains for
            # t15..t12 pre-run their high-si segments interleaved with the
            # last S rows to keep the PE fed while ACT chews the big exps.
            def o_order(tj):
                return (
                    list(range(tj, 7, -1))
                    + list(range(min(tj, 3), -1, -1))
                    + list(range(4, min(tj, 7) + 1))
                )

            o_tiles = {}

            def o_alloc(tj, pool, tag):
                o_tiles[tj] = o_tiles.get(tj) or pool.tile(
                    [128, 129], F32, tag=tag, name=f"o_ps{tj}"
                )

            def o_seg(tj, seg):
                full = o_order(tj)
                for si in seg:
                    nc.tensor.matmul(
                        o_tiles[tj][:],
                        p_rows[si][:, (tj - si) * 128 : (tj - si + 1) * 128],
                        v_rows[si][:],
                        start=(si == full[0]),
                        stop=(si == full[-1]),
                    )

            # ---- G4: diagonal chunks complete rows 0..3 (gates for every O
            # chain), then rows 4..7 (the last gates); v rows and O
            # pre-bodies fill the PE between the exp-paced S matmuls ----
            s_chunks(0, 0, 512)
            v_row(7)
            s_chunks(1, 128, 512)
            v_row(6)
            s_chunks(2, 256, 512)
            v_row(5)
            s_chunks(3, 384, 512)
            v_row(4)
            s_row(4)
            v_row(3)
            s_row(5)
            v_row(2)
            s_row(6)
            v_row(1)
            s_row(7)
            v_row(0)
            o_alloc(15, acc_ps, "acc")
            o_seg(15, list(range(15, 7, -1)) + [3, 2, 1, 0])
            o_alloc(14, proj_ps, "proj")
            o_seg(14, list(range(14, 7, -1)) + [3, 2, 1, 0])
            o_alloc(13, acc_ps, "acc")
            o_seg(13, list(range(13, 7, -1)) + [3, 2, 1, 0])
            o_alloc(12, proj_ps, "proj")
            o_seg(12, list(range(12, 7, -1)) + [3, 2, 1, 0])

            def epilogue(tj, o_ps, last=False):
                recip = ep_pool.tile([128, 1], F32, tag="recip", bufs=6)
                nc.vector.reciprocal(recip[:], o_ps[:, 128:129])
                out_sb = ep_pool.tile([128, 128], F32, tag="outsb", bufs=12)
                if tj % 2 == 0 and tj >= 4 and not last:
                    nc.scalar.activation(
                        out_sb[:], o_ps[:, 0:128], AF.Identity,
                        scale=recip[:, 0:1],
                    )
                else:
                    nc.vector.tensor_scalar_mul(
                        out_sb[:], o_ps[:, 0:128], recip[:, 0:1]
                    )
                dma_eng = nc.sync if (tj % 2 == 1 or last) else nc.gpsimd
                dma_eng.dma_start(
                    out_d[tj * 128 : (tj + 1) * 128, :], out_sb[:]
                )

            # ---- O finale. Tiny chains t3..t0 first (ungated: rows <= 3
            # ready long before the rows 4..7 exps that gate the fins) on
            # the drained S ring, then the fins, then the big chains. ----
            for tj in (3, 2, 1, 0):
                o_alloc(tj, s_ps_pool, "sps")
                o_seg(tj, o_order(tj))
                epilogue(tj, o_tiles[tj])
            for tj in (15, 14, 13, 12):
                o_seg(tj, [4, 5, 6, 7])
                epilogue(tj, o_tiles[tj])
            for i, tj in enumerate((11, 10, 9, 8, 7, 6, 5, 4)):
                pool = acc_ps if i % 2 == 0 else proj_ps
                o_alloc(tj, pool, "acc" if i % 2 == 0 else "proj")
                o_seg(tj, o_order(tj))
                epilogue(tj, o_tiles[tj], last=(tj == 4))

    nc.compile()
    return nc


_NC = None


def _get_nc():
    global _NC
    if _NC is None:
        _NC = build_nc()
    return _NC


def _make_in_maps(x, Wq, bq, Wk, bk, Wv, bv):
    bf = ml_dtypes.bfloat16

    def chunk_w(w):  # [1024, 128] -> [128, 8, 128] (partition, d-chunk, h)
        return np.ascontiguousarray(
            w.astype(bf).reshape(ND, 128, H).transpose(1, 0, 2)
        )

    shared = {
        "wq": chunk_w(Wq),
        "wk": chunk_w(Wk),
        "wv": chunk_w(Wv),
        "bias": np.ascontiguousarray(
            np.stack([bq, bk], axis=1).astype(np.float32)
        ),
        "mask": np.triu(np.ones((128, 128), dtype=np.float32)).astype(bf),
        "bvb": np.ascontiguousarray(
            np.broadcast_to(bv.astype(np.float32), (128, 128))
        ),
    }
    in_maps = []
    for i in range(B):
        m = dict(shared)
        xT = x[i].astype(bf).T  # [1024, 2048]
        # [pair j, dc-in-pair, part, half, 1024] -> [j, part, dc-in-pair*1024]
        xTc = xT.reshape(4, 2, 128, 2, 1024).transpose(0, 2, 1, 3, 4)
        m["xb"] = np.ascontiguousarray(xTc[:, :, :, 1, :]).reshape(4, 128, 2048)
        m["xa"] = np.ascontiguousarray(xTc[:, :, :, 0, :]).reshape(4, 128, 2048)
        in_maps.append(m)
    return in_maps


def _run(inputs, trace=False, **kw):
    nc = _get_nc()
    in_maps = _make_in_maps(**inputs)
    res = run_bass_kernel_spmd(nc, in_maps, core_ids=list(range(B)), trace=trace, **kw)
    out = np.stack([res.results[i]["out"] for i in range(B)], axis=0)
    return out.astype(np.float32), res


def kernel(x, Wq, bq, Wk, bk, Wv, bv):
    out, _ = _run(dict(x=x, Wq=Wq, bq=bq, Wk=Wk, bk=bk, Wv=Wv, bv=bv))
    return out


# revision 59
# speedup vs baseline: 1.0295x; 1.0295x over previous
"""Single-head causal attention on 8 TRN2 NeuronCores.

Problem: x[8, 2048, 1024] f32; Wq/Wk/Wv[1024, 128]; bq/bk/bv[128].
  q = x@Wq+bq; k = x@Wk+bk; v = x@Wv+bv
  scores[b,t,s] = k[b,t,:].q[b,s,:] / sqrt(128), causal (s<=t), softmax over s
  out = weights @ v   -> [8, 2048, 128] f32

Sharding: data-parallel over batch, one batch element per core. No collectives.

Per-core algorithm (T=2048, D=1024, H=128), matmuls in bf16. Design notes:
  - scores computed TRANSPOSED: S_T[s, t] = qT.T @ kT, so P_T = exp(S_T) is
    directly the stationary operand of out[t, 129] = P_T.T @ v_aug; the ones
    column of v_aug yields the softmax denominator for free.
  - bv is folded into the v rows (v' = v + bv): softmax weights sum to 1, so
    out/denom + bv == (P@(v+bv))/denom.  Kills the epilogue bias add.
  - x is loaded as dc-pair chunks [128, 2048] (4KB rows: descriptors below
    4KB run at reduced HBM efficiency), B column-half first; DMA triggers
    split between the SP and ACT hardware DGE queues (each trigger costs
    ~600-700ns of queue time) plus gpsimd SWDGE for half the output tiles.
  - ONE set of PSUM pools lives for the whole kernel (no pool-handoff
    barriers): proj ring (2 banks) + S ring (3) + v/O accumulator ring (3);
    O tiles also recycle the proj ring once projections finish.
  - schedule: G1 k3+v15/v14 track the x pair DMAs, q3 chain after (wq lands
    late); G2 S rows 15..8 at their earliest deps (k3/q3, then k2/q2) keep
    the ACT exp stream fed from ~22us; G3 runs all four A-half proj chains
    the moment the A half lands, with rows 0..3's high chunks dropping in
    as q0/k1 resolve; G4 finishes rows 0..3 (diag) then rows 4..7 -- the
    last gates -- with v-row chains as PE filler between exp-paced S
    matmuls, then O chains t15..t12 pre-run all but their si 4..7 tails.
  - O accumulation chains order si so chains end on rows 7..4 (exp'd last);
    epilogue = DVE reciprocal + DVE/ACT multiply, out-DMAs alternate
    SP/gpsimd queues; tiny chains t3..t0 run early in the finale and the
    final tile takes the fastest epilogue path (DVE + SP).
"""

import math

import ml_dtypes
import numpy as np

import concourse.bass as bass
import concourse.mybir as mybir
import concourse.tile as tile
from concourse import bacc
from concourse.bass_utils import run_bass_kernel_spmd

B, T, D, H = 8, 2048, 1024, 128
NT = T // 128          # 16 t/s tiles
ND = D // 128          # 8 contraction chunks
SCALE = 1.0 / math.sqrt(H)

F32 = mybir.dt.float32
BF16 = mybir.dt.bfloat16
AF = mybir.ActivationFunctionType


def build_nc():
    nc = bacc.Bacc(
        "TRN2",
        target_bir_lowering=False,
        debug=False,
        num_devices=8,
    )

    # x[b].T split into 4 dc-pair chunks x 2 column halves: B half = cols
    # 1024:2048 (consumed first), A half = cols 0:1024. Pair chunk j holds
    # d-chunks 2j and 2j+1 side by side: [128, 2048] with 4KB contiguous
    # rows (DMA descriptors below 4KB run at reduced HBM efficiency).
    xb_d = nc.dram_tensor("xb", [4, 128, 2048], BF16, kind="ExternalInput")
    xa_d = nc.dram_tensor("xa", [4, 128, 2048], BF16, kind="ExternalInput")
    w_d = {
        p: nc.dram_tensor(f"w{p}", [128, ND, H], BF16, kind="ExternalInput")
        for p in ("q", "k", "v")
    }
    bias_d = nc.dram_tensor("bias", [H, 2], F32, kind="ExternalInput")
    mask_d = nc.dram_tensor("mask", [128, 128], BF16, kind="ExternalInput")
    bvb_d = nc.dram_tensor("bvb", [128, 128], F32, kind="ExternalInput")
    out_d = nc.dram_tensor("out", [T, H], F32, kind="ExternalOutput")

    with tile.TileContext(nc) as tc:
        with (
            tc.tile_pool(name="const", bufs=1) as const_pool,
            tc.tile_pool(name="x", bufs=1) as x_pool,
            tc.tile_pool(name="qk", bufs=1) as qk_pool,
            tc.tile_pool(name="vrows", bufs=1) as v_pool,
            tc.tile_pool(name="prows", bufs=1) as p_pool,
            tc.tile_pool(name="eps", bufs=3) as ep_pool,
            tc.tile_pool(name="projps", bufs=2, space="PSUM") as proj_ps,
            tc.tile_pool(name="sps", bufs=3, space="PSUM") as s_ps_pool,
            tc.tile_pool(name="accps", bufs=3, space="PSUM") as acc_ps,
        ):
            w_sb = {}
            for p in ("q", "k", "v"):
                w_sb[p] = const_pool.tile(
                    [128, ND, H], BF16, tag=f"w{p}", name=f"w{p}_sb"
                )
            bias_sb = const_pool.tile([128, 2], F32, tag="bias")
            mask_sb = const_pool.tile([128, 128], BF16, tag="mask")
            bvb_sb = const_pool.tile([128, 128], F32, tag="bvb")
            xh = {
                "b": [x_pool.tile([128, 2048], BF16, tag=f"xb{j}", name=f"xb{j}_sb")
                      for j in range(4)],
                "a": [x_pool.tile([128, 2048], BF16, tag=f"xa{j}", name=f"xa{j}_sb")
                      for j in range(4)],
            }

            def x_ap(half, dc, off, w):
                # column slice [off, off+w) of d-chunk dc within the half
                return xh[half][dc // 2][:, (dc % 2) * 1024 + off : (dc % 2) * 1024 + off + w]

            # ---- input DMAs ----
            # Transfers drain roughly in program order: x pair 0 first so the
            # first projection matmuls start ASAP; weights interleaved right
            # behind; A half after B half (not needed until ~mid-kernel).
            nc.scalar.dma_start(w_sb["k"][:], w_d["k"][:])
            nc.scalar.dma_start(w_sb["v"][:], w_d["v"][:])
            nc.sync.dma_start(xh["b"][0][:], xb_d[0, :, :])
            nc.sync.dma_start(xh["b"][1][:], xb_d[1, :, :])
            nc.scalar.dma_start(w_sb["q"][:], w_d["q"][:])
            nc.scalar.dma_start(bias_sb[:], bias_d[:])
            nc.sync.dma_start(xh["b"][2][:], xb_d[2, :, :])
            nc.scalar.dma_start(bvb_sb[:], bvb_d[:])
            nc.scalar.dma_start(mask_sb[:], mask_d[:])
            nc.sync.dma_start(xh["b"][3][:], xb_d[3, :, :])
            for j in range(4):
                nc.sync.dma_start(xh["a"][j][:], xa_d[j, :, :])
            # pre-warm the ACT exp table while DMAs land
            warm = const_pool.tile([128, 1], F32, tag="warm")
            nc.scalar.activation(warm[:], bias_sb[:, 0:1], AF.Exp, scale=0.0)
            # pre-warm the PE p-state during the DMA wait: the memset is
            # dependency-free (hoisted into the preamble), so this dummy
            # matmul chain runs from ~7.5us until the first x chunk lands,
            # ramping the PE clock before the real projections start.
            wmm_in = x_pool.tile([128, 512], BF16, tag="wmm", name="warm_mm_in")
            nc.vector.memset(wmm_in[:], 0.0)
            wmm_ps = s_ps_pool.tile([128, 512], F32, tag="sps", name="warm_mm_ps")
            def warm_mms(n):
                for r in range(n):
                    nc.tensor.matmul(
                        wmm_ps[:, 0:128],
                        wmm_in[:, 0:128],
                        wmm_in[:, 0:128],
                        start=(r == 0),
                        stop=(r == n - 1),
                    )

            warm_mms(40)

            qk_sb = {"q": [None] * 4, "k": [None] * 4}
            v_rows = [None] * NT
            p_rows = [None] * NT

            def proj_chunk(p, ncol):
                """8-dc accumulation chain for projection p, global cols
                [512*ncol, 512*ncol+512); returns the psum tile."""
                half = "b" if ncol >= 2 else "a"
                off = (ncol % 2) * 512
                ps = proj_ps.tile([128, 512], F32, tag="proj", name=f"ps_{p}{ncol}")
                for dc in range(ND):
                    nc.tensor.matmul(
                        ps[:],
                        w_sb[p][:, dc, :],
                        x_ap(half, dc, off, 512),
                        start=(dc == 0),
                        stop=(dc == ND - 1),
                    )
                return ps

            def proj_copy(p, ncol, ps):
                sb_t = qk_pool.tile(
                    [128, 512], BF16, tag=f"{p}{ncol}", name=f"{p}T{ncol}_sb"
                )
                bi = 0 if p == "q" else 1
                nc.vector.tensor_scalar_add(sb_t[:], ps[:], bias_sb[:, bi : bi + 1])
                qk_sb[p][ncol] = sb_t

            def v_finish(si, vp):
                vr = v_pool.tile([128, 129], BF16, tag=f"v{si}", name=f"v{si}_sb")
                nc.vector.tensor_add(vr[:, 0:128], vp[:, 0:128], bvb_sb[:])
                nc.vector.memset(vr[:, 128:129], 1.0)
                v_rows[si] = vr

            def v_row(si):
                vp = acc_ps.tile([128, 129], F32, tag="acc", name=f"v_ps{si}")
                half = "b" if si >= 8 else "a"
                for dc in range(ND):
                    nc.tensor.matmul(
                        vp[:, 0:128],
                        x_ap(half, dc, (si % 8) * 128, 128),
                        w_sb["v"][:, dc, :],
                        start=(dc == 0),
                        stop=(dc == ND - 1),
                    )
                v_finish(si, vp)

            def s_chunks(si, c_lo, c_hi, alloc_pr=False):
                """S row si, global cols [c_lo, c_hi): matmul + exp per
                512-aligned chunk; mask if the diagonal chunk is included."""
                gc0 = si * 128
                if alloc_pr:
                    p_rows[si] = p_pool.tile(
                        [128, T - gc0], BF16, tag=f"p{si}", name=f"p{si}_sb"
                    )
                pr = p_rows[si]
                c = c_lo
                while c < c_hi:
                    ce = min(c_hi, (c // 512 + 1) * 512)
                    s_ps = s_ps_pool.tile(
                        [128, 512], F32, tag="sps", name=f"s_ps_{si}_{c}"
                    )
                    nc.tensor.matmul(
                        s_ps[:, 0 : ce - c],
                        qk_sb["q"][si // 4][:, (si % 4) * 128 : (si % 4 + 1) * 128],
                        qk_sb["k"][c // 512][:, c % 512 : c % 512 + (ce - c)],
                        start=True,
                        stop=True,
                    )
                    nc.scalar.activation(
                        pr[:, c - gc0 : ce - gc0],
                        s_ps[:, 0 : ce - c],
                        AF.Exp,
                        scale=SCALE,
                    )
                    c = ce
                if c_lo == gc0:  # diagonal chunk: causal mask (keep s <= t)
                    nc.vector.tensor_mul(pr[:, 0:128], pr[:, 0:128], mask_sb[:])

            def s_row(si):
                s_chunks(si, si * 128, T, alloc_pr=True)

            # ---- G1: {k3,k2} proj + v15/v14 track the xb pair arrivals
            # (~1.3us of PE work per ~1.4us pair: dense enough to hold the
            # PE p-state at full clock). wk/wv land first; wq lands 5th, so
            # the q chains run contiguously after, on the freed k slots. ----
            ps_k3 = proj_ps.tile([128, 512], F32, tag="proj", name="ps_k3")
            ps_k2 = proj_ps.tile([128, 512], F32, tag="proj", name="ps_k2")
            vp1 = {
                si: acc_ps.tile([128, 129], F32, tag="acc", name=f"v_ps{si}")
                for si in (15, 14)
            }
            for j in range(4):
                pair = (2 * j, 2 * j + 1)
                for dc in pair:
                    nc.tensor.matmul(
                        ps_k3[:], w_sb["k"][:, dc, :], x_ap("b", dc, 512, 512),
                        start=(dc == 0), stop=(dc == ND - 1),
                    )
                for dc in pair:
                    nc.tensor.matmul(
                        ps_k2[:], w_sb["k"][:, dc, :], x_ap("b", dc, 0, 512),
                        start=(dc == 0), stop=(dc == ND - 1),
                    )
                for si in (15, 14):
                    for dc in pair:
                        nc.tensor.matmul(
                            vp1[si][:, 0:128],
                            x_ap("b", dc, (si % 8) * 128, 128),
                            w_sb["v"][:, dc, :],
                            start=(dc == 0),
                            stop=(dc == ND - 1),
                        )
                # dependency-free filler holds the PE clock through the
                # wait for the next x pair DMA
                warm_mms(4)
            proj_copy("k", 3, ps_k3)
            proj_copy("k", 2, ps_k2)
            for si in (15, 14):
                v_finish(si, vp1[si])

            # ---- G2: q3/q2 chains on the freed slots; S rows 15..8 at
            # their earliest deps feed ACT's exp stream from ~19us ----
            ps_q3 = proj_chunk("q", 3)
            proj_copy("q", 3, ps_q3)
            s_row(15)
            s_row(14)
            ps_q2 = proj_chunk("q", 2)
            s_row(13)
            proj_copy("q", 2, ps_q2)
            s_row(12)
            v_row(13)
            s_row(11)
            v_row(12)
            s_row(10)
            v_row(11)
            s_row(9)
            v_row(10)
            s_row(8)

            # ---- G3: A-half proj chains the moment the A half lands; S
            # chunks of rows 0..3 drop in as soon as their q0/k1/k0 deps
            # resolve, keeping ACT's exp stream continuous; v rows fill ----
            ps_q0 = proj_chunk("q", 0)
            proj_copy("q", 0, ps_q0)
            ps_k1 = proj_chunk("k", 1)
            s_chunks(0, 1024, T, alloc_pr=True)
            s_chunks(1, 1024, T, alloc_pr=True)
            proj_copy("k", 1, ps_k1)
            ps_q1 = proj_chunk("q", 1)
            s_chunks(2, 1024, T, alloc_pr=True)
            s_chunks(3, 1024, T, alloc_pr=True)
            proj_copy("q", 1, ps_q1)
            ps_k0 = proj_chunk("k", 0)
            s_chunks(0, 512, 1024)
            v_row(9)
            s_chunks(1, 512, 1024)
            s_chunks(2, 512, 1024)
            proj_copy("k", 0, ps_k0)
            v_row(8)
            s_chunks(3, 512, 1024)

            # ---- O phase setup: per t-tile accumulation chains. Chain
            # order ends on rows 7..4 (exp'd last); O tiles alternate
            # between the acc ring and the now-free proj ring. Chains for
            # t15..t12 pre-run their high-si segments interleaved with the
            # last S rows to keep the PE fed while ACT chews the big exps.
            def o_order(tj):
                return (
                    list(range(tj, 7, -1))
                    + list(range(min(tj, 3), -1, -1))
                    + list(range(4, min(tj, 7) + 1))
                )

            o_tiles = {}

            def o_alloc(tj, pool, tag):
                o_tiles[tj] = o_tiles.get(tj) or pool.tile(
                    [128, 129], F32, tag=tag, name=f"o_ps{tj}"
                )

            def o_seg(tj, seg):
                full = o_order(tj)
                for si in seg:
                    nc.tensor.matmul(
                        o_tiles[tj][:],
                        p_rows[si][:, (tj - si) * 128 : (tj - si + 1) * 128],
                        v_rows[si][:],
                        start=(si == full[0]),
                        stop=(si == full[-1]),
                    )

            # ---- G4: diagonal chunks complete rows 0..3 (gates for every O
            # chain), then rows 4..7 (the last gates); v rows and O
            # pre-bodies fill the PE between the exp-paced S matmuls ----
            s_chunks(0, 0, 512)
            v_row(7)
            s_chunks(1, 128, 512)
            v_row(6)
            s_chunks(2, 256, 512)
            v_row(5)
            s_chunks(3, 384, 512)
            v_row(4)
            s_row(4)
            v_row(3)
            s_row(5)
            v_row(2)
            s_row(6)
            v_row(1)
            s_row(7)
            v_row(0)
            o_alloc(15, acc_ps, "acc")
            o_seg(15, list(range(15, 7, -1)) + [3, 2, 1, 0])
            o_alloc(14, proj_ps, "proj")
            o_seg(14, list(range(14, 7, -1)) + [3, 2, 1, 0])
            o_alloc(13, acc_ps, "acc")
            o_seg(13, list(range(13, 7, -1)) + [3, 2, 1, 0])
            o_alloc(12, proj_ps, "proj")
            o_seg(12, list(range(12, 7, -1)) + [3, 2, 1, 0])

            def epilogue(tj, o_ps, last=False):
                recip = ep_pool.tile([128, 1], F32, tag="recip", bufs=6)
                nc.vector.reciprocal(recip[:], o_ps[:, 128:129])
                out_sb = ep_pool.tile([128, 128], F32, tag="outsb", bufs=12)
                if tj % 2 == 0 and tj >= 4 and not last:
                    nc.scalar.activation(
                        out_sb[:], o_ps[:, 0:128], AF.Identity,
                        scale=recip[:, 0:1],
                    )
                else:
                    nc.vector.tensor_scalar_mul(
                        out_sb[:], o_ps[:, 0:128], recip[:, 0:1]
                    )
                dma_eng = nc.sync if (tj % 2 == 1 or last) else nc.gpsimd
                dma_eng.dma_start(
                    out_d[tj * 128 : (tj + 1) * 128, :], out_sb[:]
                )

            # ---- O finale. Tiny chains t3..t0 first (ungated: rows <= 3
            # ready long before the rows 4..7 exps that gate the fins) on
            # the drained S ring, then the fins, then the big chains. ----
            for tj in (3, 2, 1, 0):
                o_alloc(tj, s_ps_pool, "sps")
                o_seg(tj, o_order(tj))
                epilogue(tj, o_tiles[tj])
            for tj in (15, 14, 13, 12):
                o_seg(tj, [4, 5, 6, 7])
                epilogue(tj, o_tiles[tj])
            for i, tj in enumerate((11, 10, 9, 8, 7, 6, 5, 4)):
                pool = acc_ps if i % 2 == 0 else proj_ps
                o_alloc(tj, pool, "acc" if i % 2 == 0 else "proj")
                o_seg(tj, o_order(tj))
                epilogue(tj, o_tiles[tj], last=(tj == 4))

    nc.compile()
    return nc


_NC = None


def _get_nc():
    global _NC
    if _NC is None:
        _NC = build_nc()
    return _NC


def _make_in_maps(x, Wq, bq, Wk, bk, Wv, bv):
    bf = ml_dtypes.bfloat16

    def chunk_w(w):  # [1024, 128] -> [128, 8, 128] (partition, d-chunk, h)
        return np.ascontiguousarray(
            w.astype(bf).reshape(ND, 128, H).transpose(1, 0, 2)
        )

    shared = {
        "wq": chunk_w(Wq),
        "wk": chunk_w(Wk),
        "wv": chunk_w(Wv),
        "bias": np.ascontiguousarray(
            np.stack([bq, bk], axis=1).astype(np.float32)
        ),
        "mask": np.triu(np.ones((128, 128), dtype=np.float32)).astype(bf),
        "bvb": np.ascontiguousarray(
            np.broadcast_to(bv.astype(np.float32), (128, 128))
        ),
    }
    in_maps = []
    for i in range(B):
        m = dict(shared)
        xT = x[i].astype(bf).T  # [1024, 2048]
        # [pair j, dc-in-pair, part, half, 1024] -> [j, part, dc-in-pair*1024]
        xTc = xT.reshape(4, 2, 128, 2, 1024).transpose(0, 2, 1, 3, 4)
        m["xb"] = np.ascontiguousarray(xTc[:, :, :, 1, :]).reshape(4, 128, 2048)
        m["xa"] = np.ascontiguousarray(xTc[:, :, :, 0, :]).reshape(4, 128, 2048)
        in_maps.append(m)
    return in_maps


def _run(inputs, trace=False, **kw):
    nc = _get_nc()
    in_maps = _make_in_maps(**inputs)
    res = run_bass_kernel_spmd(nc, in_maps, core_ids=list(range(B)), trace=trace, **kw)
    out = np.stack([res.results[i]["out"] for i in range(B)], axis=0)
    return out.astype(np.float32), res


def kernel(x, Wq, bq, Wk, bk, Wv, bv):
    out, _ = _run(dict(x=x, Wq=Wq, bq=bq, Wk=Wk, bk=bk, Wv=Wv, bv=bv))
    return out


# revision 60
# speedup vs baseline: 1.1929x; 1.1587x over previous
"""Single-head causal attention on 8 TRN2 NeuronCores.

Problem: x[8, 2048, 1024] f32; Wq/Wk/Wv[1024, 128]; bq/bk/bv[128].
  q = x@Wq+bq; k = x@Wk+bk; v = x@Wv+bv
  scores[b,t,s] = k[b,t,:].q[b,s,:] / sqrt(128), causal (s<=t), softmax over s
  out = weights @ v   -> [8, 2048, 128] f32

Sharding: data-parallel over batch, one batch element per core. No collectives.

Per-core algorithm (T=2048, D=1024, H=128), matmuls in bf16. Design notes:
  - scores computed TRANSPOSED: S_T[s, t] = qT.T @ kT, so P_T = exp(S_T) is
    directly the stationary operand of out[t, 129] = P_T.T @ v_aug; the ones
    column of v_aug yields the softmax denominator for free.
  - bv is folded into the v rows (v' = v + bv): softmax weights sum to 1, so
    out/denom + bv == (P@(v+bv))/denom.  Kills the epilogue bias add.
  - x is loaded as dc-pair chunks [128, 2048] (4KB rows: descriptors below
    4KB run at reduced HBM efficiency), B column-half first; DMA triggers
    split between the SP and ACT hardware DGE queues (each trigger costs
    ~600-700ns of queue time) plus gpsimd SWDGE for half the output tiles.
  - ONE set of PSUM pools lives for the whole kernel (no pool-handoff
    barriers): proj ring (2 banks) + S ring (3) + v/O accumulator ring (3);
    O tiles also recycle the proj ring once projections finish.
  - schedule: G1 k3+v15/v14 track the x pair DMAs, q3 chain after (wq lands
    late); G2 S rows 15..8 at their earliest deps (k3/q3, then k2/q2) keep
    the ACT exp stream fed from ~22us; G3 runs all four A-half proj chains
    the moment the A half lands, with rows 0..3's high chunks dropping in
    as q0/k1 resolve; G4 finishes rows 0..3 (diag) then rows 4..7 -- the
    last gates -- with v-row chains as PE filler between exp-paced S
    matmuls, then O chains t15..t12 pre-run all but their si 4..7 tails.
  - O accumulation chains order si so chains end on rows 7..4 (exp'd last);
    epilogue = DVE reciprocal + DVE/ACT multiply, out-DMAs alternate
    SP/gpsimd queues; tiny chains t3..t0 run early in the finale and the
    final tile takes the fastest epilogue path (DVE + SP).
"""

import math

import ml_dtypes
import numpy as np

import concourse.bass as bass
import concourse.mybir as mybir
import concourse.tile as tile
from concourse import bacc
from concourse.bass_utils import run_bass_kernel_spmd

B, T, D, H = 8, 2048, 1024, 128
NT = T // 128          # 16 t/s tiles
ND = D // 128          # 8 contraction chunks
SCALE = 1.0 / math.sqrt(H)

F32 = mybir.dt.float32
BF16 = mybir.dt.bfloat16
AF = mybir.ActivationFunctionType


def build_nc():
    nc = bacc.Bacc(
        "TRN2",
        target_bir_lowering=False,
        debug=False,
        num_devices=8,
    )

    # x[b].T split into 4 dc-pair chunks x 2 column halves: B half = cols
    # 1024:2048 (consumed first), A half = cols 0:1024. Pair chunk j holds
    # d-chunks 2j and 2j+1 side by side: [128, 2048] with 4KB contiguous
    # rows (DMA descriptors below 4KB run at reduced HBM efficiency).
    xb_d = nc.dram_tensor("xb", [4, 128, 2048], BF16, kind="ExternalInput")
    xa_d = nc.dram_tensor("xa", [4, 128, 2048], BF16, kind="ExternalInput")
    w_d = {
        p: nc.dram_tensor(f"w{p}", [128, ND, H], BF16, kind="ExternalInput")
        for p in ("q", "k", "v")
    }
    bias_d = nc.dram_tensor("bias", [H, 2], F32, kind="ExternalInput")
    mask_d = nc.dram_tensor("mask", [128, 128], BF16, kind="ExternalInput")
    bvb_d = nc.dram_tensor("bvb", [128, 128], F32, kind="ExternalInput")
    out_d = nc.dram_tensor("out", [T, H], F32, kind="ExternalOutput")

    with tile.TileContext(nc) as tc:
        with (
            tc.tile_pool(name="const", bufs=1) as const_pool,
            tc.tile_pool(name="x", bufs=1) as x_pool,
            tc.tile_pool(name="qk", bufs=1) as qk_pool,
            tc.tile_pool(name="vrows", bufs=1) as v_pool,
            tc.tile_pool(name="prows", bufs=1) as p_pool,
            tc.tile_pool(name="eps", bufs=3) as ep_pool,
            tc.tile_pool(name="projps", bufs=2, space="PSUM") as proj_ps,
            tc.tile_pool(name="sps", bufs=3, space="PSUM") as s_ps_pool,
            tc.tile_pool(name="accps", bufs=3, space="PSUM") as acc_ps,
        ):
            w_sb = {}
            for p in ("q", "k", "v"):
                w_sb[p] = const_pool.tile(
                    [128, ND, H], BF16, tag=f"w{p}", name=f"w{p}_sb"
                )
            bias_sb = const_pool.tile([128, 2], F32, tag="bias")
            mask_sb = const_pool.tile([128, 128], BF16, tag="mask")
            bvb_sb = const_pool.tile([128, 128], F32, tag="bvb")
            xh = {
                "b": [x_pool.tile([128, 2048], BF16, tag=f"xb{j}", name=f"xb{j}_sb")
                      for j in range(4)],
                "a": [x_pool.tile([128, 2048], BF16, tag=f"xa{j}", name=f"xa{j}_sb")
                      for j in range(4)],
            }

            def x_ap(half, dc, off, w):
                # column slice [off, off+w) of d-chunk dc within the half
                return xh[half][dc // 2][:, (dc % 2) * 1024 + off : (dc % 2) * 1024 + off + w]

            # ---- input DMAs ----
            # Transfers drain roughly in program order: x pair 0 first so the
            # first projection matmuls start ASAP; weights interleaved right
            # behind; A half after B half (not needed until ~mid-kernel).
            nc.scalar.dma_start(w_sb["k"][:], w_d["k"][:])
            nc.scalar.dma_start(w_sb["v"][:], w_d["v"][:])
            nc.sync.dma_start(xh["b"][0][:], xb_d[0, :, :])
            nc.sync.dma_start(xh["b"][1][:], xb_d[1, :, :])
            nc.scalar.dma_start(w_sb["q"][:], w_d["q"][:])
            nc.scalar.dma_start(bias_sb[:], bias_d[:])
            nc.sync.dma_start(xh["b"][2][:], xb_d[2, :, :])
            nc.scalar.dma_start(bvb_sb[:], bvb_d[:])
            nc.scalar.dma_start(mask_sb[:], mask_d[:])
            nc.sync.dma_start(xh["b"][3][:], xb_d[3, :, :])
            for j in range(4):
                nc.sync.dma_start(xh["a"][j][:], xa_d[j, :, :])
            # pre-warm the ACT exp table while DMAs land
            warm = const_pool.tile([128, 1], F32, tag="warm")
            nc.scalar.activation(warm[:], bias_sb[:, 0:1], AF.Exp, scale=0.0)
            # pre-warm the PE p-state during the DMA wait: the memset is
            # dependency-free (hoisted into the preamble), so this dummy
            # matmul chain runs from ~7.5us until the first x chunk lands,
            # ramping the PE clock before the real projections start.
            wmm_in = x_pool.tile([128, 512], BF16, tag="wmm", name="warm_mm_in")
            nc.vector.memset(wmm_in[:], 0.0)
            wmm_ps = s_ps_pool.tile([128, 512], F32, tag="sps", name="warm_mm_ps")
            def warm_mms(n):
                for r in range(n):
                    nc.tensor.matmul(
                        wmm_ps[:, 0:128],
                        wmm_in[:, 0:128],
                        wmm_in[:, 0:128],
                        start=(r == 0),
                        stop=(r == n - 1),
                    )

            warm_mms(44)

            qk_sb = {"q": [None] * 4, "k": [None] * 4}
            v_rows = [None] * NT
            p_rows = [None] * NT

            def proj_chunk(p, ncol):
                """8-dc accumulation chain for projection p, global cols
                [512*ncol, 512*ncol+512); returns the psum tile."""
                half = "b" if ncol >= 2 else "a"
                off = (ncol % 2) * 512
                ps = proj_ps.tile([128, 512], F32, tag="proj", name=f"ps_{p}{ncol}")
                for dc in range(ND):
                    nc.tensor.matmul(
                        ps[:],
                        w_sb[p][:, dc, :],
                        x_ap(half, dc, off, 512),
                        start=(dc == 0),
                        stop=(dc == ND - 1),
                    )
                return ps

            def proj_copy(p, ncol, ps):
                sb_t = qk_pool.tile(
                    [128, 512], BF16, tag=f"{p}{ncol}", name=f"{p}T{ncol}_sb"
                )
                bi = 0 if p == "q" else 1
                nc.vector.tensor_scalar_add(sb_t[:], ps[:], bias_sb[:, bi : bi + 1])
                qk_sb[p][ncol] = sb_t

            def v_finish(si, vp):
                vr = v_pool.tile([128, 129], BF16, tag=f"v{si}", name=f"v{si}_sb")
                nc.vector.tensor_add(vr[:, 0:128], vp[:, 0:128], bvb_sb[:])
                nc.vector.memset(vr[:, 128:129], 1.0)
                v_rows[si] = vr

            def v_row(si):
                vp = acc_ps.tile([128, 129], F32, tag="acc", name=f"v_ps{si}")
                half = "b" if si >= 8 else "a"
                for dc in range(ND):
                    nc.tensor.matmul(
                        vp[:, 0:128],
                        x_ap(half, dc, (si % 8) * 128, 128),
                        w_sb["v"][:, dc, :],
                        start=(dc == 0),
                        stop=(dc == ND - 1),
                    )
                v_finish(si, vp)

            def s_chunks(si, c_lo, c_hi, alloc_pr=False):
                """S row si, global cols [c_lo, c_hi): matmul + exp per
                512-aligned chunk; mask if the diagonal chunk is included."""
                gc0 = si * 128
                if alloc_pr:
                    p_rows[si] = p_pool.tile(
                        [128, T - gc0], BF16, tag=f"p{si}", name=f"p{si}_sb"
                    )
                pr = p_rows[si]
                c = c_lo
                while c < c_hi:
                    ce = min(c_hi, (c // 512 + 1) * 512)
                    s_ps = s_ps_pool.tile(
                        [128, 512], F32, tag="sps", name=f"s_ps_{si}_{c}"
                    )
                    nc.tensor.matmul(
                        s_ps[:, 0 : ce - c],
                        qk_sb["q"][si // 4][:, (si % 4) * 128 : (si % 4 + 1) * 128],
                        qk_sb["k"][c // 512][:, c % 512 : c % 512 + (ce - c)],
                        start=True,
                        stop=True,
                    )
                    nc.scalar.activation(
                        pr[:, c - gc0 : ce - gc0],
                        s_ps[:, 0 : ce - c],
                        AF.Exp,
                        scale=SCALE,
                    )
                    c = ce
                if c_lo == gc0:  # diagonal chunk: causal mask (keep s <= t)
                    nc.vector.tensor_mul(pr[:, 0:128], pr[:, 0:128], mask_sb[:])

            def s_row(si):
                s_chunks(si, si * 128, T, alloc_pr=True)

            # ---- G1: {k3,k2} proj + v15/v14 track the xb pair arrivals
            # (~1.3us of PE work per ~1.4us pair: dense enough to hold the
            # PE p-state at full clock). wk/wv land first; wq lands 5th, so
            # the q chains run contiguously after, on the freed k slots. ----
            ps_k3 = proj_ps.tile([128, 512], F32, tag="proj", name="ps_k3")
            ps_k2 = proj_ps.tile([128, 512], F32, tag="proj", name="ps_k2")
            vp1 = {
                si: acc_ps.tile([128, 129], F32, tag="acc", name=f"v_ps{si}")
                for si in (15, 14)
            }
            for j in range(4):
                pair = (2 * j, 2 * j + 1)
                for dc in pair:
                    nc.tensor.matmul(
                        ps_k3[:], w_sb["k"][:, dc, :], x_ap("b", dc, 512, 512),
                        start=(dc == 0), stop=(dc == ND - 1),
                    )
                for dc in pair:
                    nc.tensor.matmul(
                        ps_k2[:], w_sb["k"][:, dc, :], x_ap("b", dc, 0, 512),
                        start=(dc == 0), stop=(dc == ND - 1),
                    )
                for si in (15, 14):
                    for dc in pair:
                        nc.tensor.matmul(
                            vp1[si][:, 0:128],
                            x_ap("b", dc, (si % 8) * 128, 128),
                            w_sb["v"][:, dc, :],
                            start=(dc == 0),
                            stop=(dc == ND - 1),
                        )
                # dependency-free filler holds the PE clock through the
                # wait for the next x pair DMA
                warm_mms(6)
            proj_copy("k", 3, ps_k3)
            proj_copy("k", 2, ps_k2)
            for si in (15, 14):
                v_finish(si, vp1[si])

            # ---- G2: q3/q2 chains on the freed slots; S rows 15..8 at
            # their earliest deps feed ACT's exp stream from ~19us ----
            ps_q3 = proj_chunk("q", 3)
            proj_copy("q", 3, ps_q3)
            s_row(15)
            s_row(14)
            ps_q2 = proj_chunk("q", 2)
            s_row(13)
            proj_copy("q", 2, ps_q2)
            s_row(12)
            v_row(13)
            s_row(11)
            v_row(12)
            s_row(10)
            v_row(11)
            s_row(9)
            v_row(10)
            s_row(8)

            # ---- G3: A-half proj chains the moment the A half lands; S
            # chunks of rows 0..3 drop in as soon as their q0/k1/k0 deps
            # resolve, keeping ACT's exp stream continuous; v rows fill ----
            ps_q0 = proj_chunk("q", 0)
            proj_copy("q", 0, ps_q0)
            ps_k1 = proj_chunk("k", 1)
            s_chunks(0, 1024, T, alloc_pr=True)
            s_chunks(1, 1024, T, alloc_pr=True)
            proj_copy("k", 1, ps_k1)
            ps_q1 = proj_chunk("q", 1)
            s_chunks(2, 1024, T, alloc_pr=True)
            s_chunks(3, 1024, T, alloc_pr=True)
            proj_copy("q", 1, ps_q1)
            ps_k0 = proj_chunk("k", 0)
            s_chunks(0, 512, 1024)
            v_row(9)
            s_chunks(1, 512, 1024)
            s_chunks(2, 512, 1024)
            proj_copy("k", 0, ps_k0)
            v_row(8)
            s_chunks(3, 512, 1024)

            # ---- O phase setup: per t-tile accumulation chains. Chain
            # order ends on rows 7..4 (exp'd last); O tiles alternate
            # between the acc ring and the now-free proj ring. Chains for
            # t15..t12 pre-run their high-si segments interleaved with the
            # last S rows to keep the PE fed while ACT chews the big exps.
            def o_order(tj):
                return (
                    list(range(tj, 7, -1))
                    + list(range(min(tj, 3), -1, -1))
                    + list(range(4, min(tj, 7) + 1))
                )

            o_tiles = {}

            def o_alloc(tj, pool, tag):
                o_tiles[tj] = o_tiles.get(tj) or pool.tile(
                    [128, 129], F32, tag=tag, name=f"o_ps{tj}"
                )

            def o_seg(tj, seg):
                full = o_order(tj)
                for si in seg:
                    nc.tensor.matmul(
                        o_tiles[tj][:],
                        p_rows[si][:, (tj - si) * 128 : (tj - si + 1) * 128],
                        v_rows[si][:],
                        start=(si == full[0]),
                        stop=(si == full[-1]),
                    )

            # ---- G4: diagonal chunks complete rows 0..3 (gates for every O
            # chain), then rows 4..7 (the last gates); v rows and O
            # pre-bodies fill the PE between the exp-paced S matmuls ----
            s_chunks(0, 0, 512)
            v_row(7)
            s_chunks(1, 128, 512)
            v_row(6)
            s_chunks(2, 256, 512)
            v_row(5)
            s_chunks(3, 384, 512)
            v_row(4)
            s_row(4)
            v_row(3)
            s_row(5)
            v_row(2)
            s_row(6)
            v_row(1)
            s_row(7)
            v_row(0)
            o_alloc(15, acc_ps, "acc")
            o_seg(15, list(range(15, 7, -1)) + [3, 2, 1, 0])
            o_alloc(14, proj_ps, "proj")
            o_seg(14, list(range(14, 7, -1)) + [3, 2, 1, 0])
            o_alloc(13, acc_ps, "acc")
            o_seg(13, list(range(13, 7, -1)) + [3, 2, 1, 0])
            o_alloc(12, proj_ps, "proj")
            o_seg(12, list(range(12, 7, -1)) + [3, 2, 1, 0])

            def epilogue(tj, o_ps, last=False):
                recip = ep_pool.tile([128, 1], F32, tag="recip", bufs=6)
                nc.vector.reciprocal(recip[:], o_ps[:, 128:129])
                out_sb = ep_pool.tile([128, 128], F32, tag="outsb", bufs=12)
                if tj % 2 == 0 and tj >= 4 and not last:
                    nc.scalar.activation(
                        out_sb[:], o_ps[:, 0:128], AF.Identity,
                        scale=recip[:, 0:1],
                    )
                else:
                    nc.vector.tensor_scalar_mul(
                        out_sb[:], o_ps[:, 0:128], recip[:, 0:1]
                    )
                dma_eng = nc.sync if (tj % 2 == 1 or last) else nc.gpsimd
                dma_eng.dma_start(
                    out_d[tj * 128 : (tj + 1) * 128, :], out_sb[:]
                )

            # ---- O finale. Tiny chains t3..t0 first (ungated: rows <= 3
            # ready long before the rows 4..7 exps that gate the fins) on
            # the drained S ring, then the fins, then the big chains. ----
            for tj in (3, 2, 1, 0):
                o_alloc(tj, s_ps_pool, "sps")
                o_seg(tj, o_order(tj))
                epilogue(tj, o_tiles[tj])
            # filler on a free acc slot bridges the wait for the last
            # exps (rows 6/7) that gate the pre-run chains' tails
            wmm2_ps = acc_ps.tile([128, 129], F32, tag="acc", name="warm2_ps")
            for r in range(6):
                nc.tensor.matmul(
                    wmm2_ps[:, 0:128], wmm_in[:, 0:128], wmm_in[:, 0:128],
                    start=(r == 0), stop=(r == 5),
                )
            for tj in (15, 14, 13, 12):
                o_seg(tj, [4, 5, 6, 7])
                epilogue(tj, o_tiles[tj])
            for i, tj in enumerate((11, 10, 9, 8, 7, 6, 5, 4)):
                pool = acc_ps if i % 2 == 0 else proj_ps
                o_alloc(tj, pool, "acc" if i % 2 == 0 else "proj")
                o_seg(tj, o_order(tj))
                epilogue(tj, o_tiles[tj], last=(tj == 4))

    nc.compile()
    return nc


_NC = None


def _get_nc():
    global _NC
    if _NC is None:
        _NC = build_nc()
    return _NC


def _make_in_maps(x, Wq, bq, Wk, bk, Wv, bv):
    bf = ml_dtypes.bfloat16

    def chunk_w(w):  # [1024, 128] -> [128, 8, 128] (partition, d-chunk, h)
        return np.ascontiguousarray(
            w.astype(bf).reshape(ND, 128, H).transpose(1, 0, 2)
        )

    shared = {
        "wq": chunk_w(Wq),
        "wk": chunk_w(Wk),
        "wv": chunk_w(Wv),
        "bias": np.ascontiguousarray(
            np.stack([bq, bk], axis=1).astype(np.float32)
        ),
        "mask": np.triu(np.ones((128, 128), dtype=np.float32)).astype(bf),
        "bvb": np.ascontiguousarray(
            np.broadcast_to(bv.astype(np.float32), (128, 128))
        ),
    }
    in_maps = []
    for i in range(B):
        m = dict(shared)
        xT = x[i].astype(bf).T  # [1024, 2048]
        # [pair j, dc-in-pair, part, half, 1024] -> [j, part, dc-in-pair*1024]
        xTc = xT.reshape(4, 2, 128, 2, 1024).transpose(0, 2, 1, 3, 4)
        m["xb"] = np.ascontiguousarray(xTc[:, :, :, 1, :]).reshape(4, 128, 2048)
        m["xa"] = np.ascontiguousarray(xTc[:, :, :, 0, :]).reshape(4, 128, 2048)
        in_maps.append(m)
    return in_maps


def _run(inputs, trace=False, **kw):
    nc = _get_nc()
    in_maps = _make_in_maps(**inputs)
    res = run_bass_kernel_spmd(nc, in_maps, core_ids=list(range(B)), trace=trace, **kw)
    out = np.stack([res.results[i]["out"] for i in range(B)], axis=0)
    return out.astype(np.float32), res


def kernel(x, Wq, bq, Wk, bk, Wv, bv):
    out, _ = _run(dict(x=x, Wq=Wq, bq=bq, Wk=Wk, bk=bk, Wv=Wv, bv=bv))
    return out
